# revision 34
# baseline (speedup 1.0000x reference)
"""Multi-head attention block (q/k/v projections + softmax attention +
out-projection) distributed over 8 TRN2 NeuronCores.

Sharding (head-split / tensor-parallel within batch pairs): core c handles
batch b = c//2 and HEADS 4h..4h+3 (h = c%2) over the FULL 2048 query rows,
so the k/v projections are computed once per (batch, head) with no
duplication (the old query-split scheme recomputed kh/vh on both cores of a
pair).  After attention, the cores of a pair exchange attention outputs via
a tiny per-pair AllGather (0.25 MB) and each core runs the out-projection
for its half of the query rows over all 8 heads.

SPMD uniformity: the same program runs on all 8 cores.  Per-core
asymmetries are folded into the HOST-side input layout:
  - q rows are permuted to [own half | peer half], so "my output rows" are
    always local columns 0:1024 of qT/attnT;
  - Wq/Wk/Wv are column-sliced to the core's own 4 heads;
  - Wo rows are permuted to [own pairs | peer pairs];
  - a 2-element flag tensor [1-h, h] selects the peer's chunk from the
    AllGather result with two tensor_scalar multiplies (no data-dependent
    addressing needed).

All matrix inputs are cast to bf16 on the host (the kernel used bf16
internally anyway), halving DMA bytes.

Per-core dataflow:
  kv/q stream in bf16 chunks -> PE transpose -> kvT/qT [model_dim, seq].
  projections (bf16 matmuls, fp32 PSUM): khT/qhT [256, 2048] for the own 2
  head-pairs, vh [k, 4*(64+1)] natural with a ones column (P@[V|1] yields
  the softmax denominator for free).
  attention per own head-pair over 4 q-blocks in order qb = [2,3,0,1]
  (PEER-half first, so the exchange never gates the tail): scores S^T[k,q]
  on fp32 PSUM (two heads row-packed via tile_position, concurrent)
  -> exp(s/8) on ScalarE -> bf16 -> PV [65, q] PSUM accumulation
  -> denominator -> reciprocal -> attnT bf16.
  The k-loop is grouped two k-tiles per phase (software-pipelined depth-2
  with block-boundary carry) so LDWEIGHTS hides; projections, transposes,
  exchange DMAs and out-projection partials ride the pipeline as PE filler.
"""

import sys

sys.path.insert(0, "/opt/trn_rl_repo")

import numpy as np

B, NQ_FULL, NK = 4, 2048, 2048
NQ = 1024          # per-core OUTPUT query rows
NQL = 2048         # per-core local query rows (own half + peer half)
DQ, DKV = 512, 768
HEADS, DH = 8, 64
INNER = 512
INNER_L = 256      # own 4 heads
DA = DH + 1        # head dim + ones column
N_CORES = 8

_cache = {}


def _build():
    import concourse.bass as bass
    import concourse.tile as tile
    from concourse import bacc, mybir

    F32 = mybir.dt.float32
    F32R = mybir.dt.float32r
    BF16 = mybir.dt.bfloat16
    EXP = mybir.ActivationFunctionType.Exp

    nc = bacc.Bacc("TRN2", target_bir_lowering=False, debug=False,
                   enable_asserts=True, num_devices=N_CORES)

    q_d = nc.dram_tensor("q", [DQ, NQL], BF16, kind="ExternalInput").ap()
    kv_d = nc.dram_tensor("kv", [DKV, NK], BF16, kind="ExternalInput").ap()
    wq_d = nc.dram_tensor("Wq", [128, 4, INNER_L], BF16, kind="ExternalInput").ap()
    wk_d = nc.dram_tensor("Wk", [128, 6, INNER_L], BF16, kind="ExternalInput").ap()
    wv_d = nc.dram_tensor("Wv", [128, 6, INNER_L], BF16, kind="ExternalInput").ap()
    wo_d = nc.dram_tensor("Wo", [128, 4, DQ], BF16, kind="ExternalInput").ap()
    bo_d = nc.dram_tensor("bo", [DQ], F32, kind="ExternalInput").ap()
    fl_d = nc.dram_tensor("flag", [2], F32, kind="ExternalInput").ap()
    out_d = nc.dram_tensor("out", [NQ, DQ], F32, kind="ExternalOutput").ap()

    MT_Q = DQ // 128      # 4
    MT_KV = DKV // 128    # 6
    ITL = INNER_L // 128  # 2 own inner tiles (= own head pairs)
    KT = NK // 128        # 16
    NT = NQ // 128        # 8 output row tiles
    G = KT // 2           # 8 attention groups (2 k-tiles each) per block

    with tile.TileContext(nc) as tc:
        with (
            tc.tile_pool(name="consts", bufs=1) as consts,
            tc.tile_pool(name="wpool", bufs=1) as wpool,
            tc.tile_pool(name="xT", bufs=1) as xT_pool,
            tc.tile_pool(name="proj", bufs=1) as proj_pool,
            tc.tile_pool(name="attnT", bufs=1) as attnT_pool,
            tc.tile_pool(name="oacc", bufs=1) as oacc_pool,
            tc.tile_pool(name="exps", bufs=7) as exps_pool,
            tc.tile_pool(name="norm", bufs=2) as norm_pool,
            tc.tile_pool(name="outs", bufs=4) as outs_pool,
            tc.tile_pool(name="blnd", bufs=2) as blnd_pool,
            tc.tile_pool(name="dram", bufs=8, space="DRAM") as dram_pool,
            tc.tile_pool(name="mm", bufs=2, space="PSUM") as ps_mm,
            tc.tile_pool(name="sc", bufs=2, space="PSUM") as ps_sc,
            tc.tile_pool(name="pv", bufs=2, space="PSUM") as ps_pv,
        ):
            from concourse.masks import make_identity
            ident = consts.tile([128, 128], F32)
            make_identity(nc, ident[:])
            ident_bf = consts.tile([128, 128], BF16)
            nc.vector.tensor_copy(ident_bf[:], ident[:])

            # HAM warmup: dependency-free matmuls while the first DMAs
            # stream, so the clock gate opens before the real prologue.
            wu = ps_sc.tile([128, 1024], F32, tag="sc", name="wu")
            for i in range(40):
                nc.tensor.matmul(wu[:, (i % 8) * 128:(i % 8 + 1) * 128],
                                 ident_bf[:], ident_bf[:],
                                 start=True, stop=True)


            wk_b = wpool.tile([128, MT_KV, INNER_L], BF16, tag="wk")
            wq_b = wpool.tile([128, MT_Q, INNER_L], BF16, tag="wq")
            wv_b = wpool.tile([128, MT_KV, INNER_L], BF16, tag="wv")
            wo_b = wpool.tile([128, 4, 512], BF16, tag="wo")
            kvT = [xT_pool.tile([128, NK], BF16, tag=f"kvT{mt}", name=f"kvT{mt}")
                   for mt in range(MT_KV)]
            qT = [xT_pool.tile([128, NQL], BF16, tag=f"qT{mt}", name=f"qT{mt}")
                  for mt in range(MT_Q)]

            # ---- projection outputs / accumulators / exchange buffers ----
            qhT = [proj_pool.tile([128, NQL], BF16, tag=f"qhT{i}", name=f"qhT{i}")
                   for i in range(ITL)]
            khT = [proj_pool.tile([128, NK], BF16, tag=f"khT{i}", name=f"khT{i}")
                   for i in range(ITL)]
            vh = [proj_pool.tile([128, 4, DA], BF16, tag=f"vh{k}", name=f"vh{k}")
                  for k in range(KT)]
            attnT = [attnT_pool.tile([128, NQL], BF16, tag=f"at{i}", name=f"at{i}")
                     for i in range(ITL)]
            attnP = [attnT_pool.tile([128, NQ], BF16, tag=f"ap{i}", name=f"ap{i}")
                     for i in range(ITL)]
            imp = [attnT_pool.tile([128, 2, NQ], BF16, tag=f"im{i}", name=f"im{i}")
                   for i in range(ITL)]
            oacc = [oacc_pool.tile([128, DQ], F32, tag=f"oa{nt}", name=f"oa{nt}")
                    for nt in range(NT)]
            cc_in = [[dram_pool.tile([128, 512], BF16, tag=f"ci{i}{j}",
                               name=f"ci{i}{j}") for j in range(2)]
                     for i in range(ITL)]
            cc_out = [[dram_pool.tile([2, 128, 512], BF16, tag=f"co{i}{j}",
                               name=f"co{i}{j}") for j in range(2)]
                      for i in range(ITL)]

            def load_w(wd, wt):
                nc.gpsimd.dma_start(wt[:], wd)

            # ---- prologue: q/kv arrive HOST-TRANSPOSED, so kvT/qT are
            # filled by plain DMA (2 KB descriptors, full throughput) with
            # no PE transposes or staging at all; the needed-first halves
            # (k 0:1024, peer q-half 1024:2048) load first
            # spread input loads over three issue queues so the
            # transfers run on parallel DMA engines
            _eng = [nc.sync, nc.scalar]

            def load_kvT(mt, half):
                if half == 0 and mt >= 4:
                    return  # loaded on the gpsimd queue in the prologue
                _eng[mt % 2].dma_start(
                    kvT[mt][:, half * 1024:(half + 1) * 1024],
                    kv_d[mt * 128:(mt + 1) * 128,
                         half * 1024:(half + 1) * 1024])

            def load_qT(mt, half):
                _eng[mt % 2].dma_start(
                    qT[mt][:, half * 1024:(half + 1) * 1024],
                    q_d[mt * 128:(mt + 1) * 128,
                        half * 1024:(half + 1) * 1024])

            load_w(wk_d, wk_b)
            for mt in range(MT_KV):
                load_kvT(mt, 0)
            # two critical first-half loads also ride gpsimd (right after
            # wk) so the three DMA engines split the first 1.5 MB
            nc.gpsimd.dma_start(kvT[4][:, 0:1024], kv_d[4 * 128:5 * 128, 0:1024])
            nc.gpsimd.dma_start(kvT[5][:, 0:1024], kv_d[5 * 128:6 * 128, 0:1024])
            load_w(wq_d, wq_b)
            for mt in range(MT_Q):
                load_qT(mt, 1)
            load_w(wv_d, wv_b)
            load_w(wo_d, wo_b)
            # late halves: not needed until mid-block-0 / block 2, so they
            # ride the gpsimd queue (ahead of the collectives) to keep the
            # sync/scalar queues free for the critical first halves
            for mt in range(MT_KV):
                nc.gpsimd.dma_start(
                    kvT[mt][:, 1024:2048],
                    kv_d[mt * 128:(mt + 1) * 128, 1024:2048])
            for mt in range(MT_Q):
                nc.gpsimd.dma_start(
                    qT[mt][:, 0:1024],
                    q_d[mt * 128:(mt + 1) * 128, 0:1024])

            # ---- constants ----
            ones1 = consts.tile([1, 64], BF16)
            nc.vector.memset(ones1[:], 1.0)
            ones4 = consts.tile([128, 4, 1], BF16)
            ones4f = consts.tile([128, 4, 1], F32)
            nc.vector.memset(ones4f[:], 1.0)
            nc.vector.tensor_copy(ones4[:], ones4f[:])
            bo_b = consts.tile([128, DQ], F32)
            nc.gpsimd.dma_start(
                out=bo_b[:],
                in_=bass.AP(tensor=bo_d.tensor, offset=bo_d.offset,
                            ap=[[0, 128]] + list(bo_d.ap)),
            )
            fl_b = consts.tile([128, 2], F32)
            nc.gpsimd.dma_start(
                out=fl_b[:],
                in_=bass.AP(tensor=fl_d.tensor, offset=fl_d.offset,
                            ap=[[0, 128]] + list(fl_d.ap)),
            )

            _khT_pp = {}

            def emit_khT_half(it, nb, half):
                # 6-MM khT unit split in two 3-MM halves so a filler slot
                # never exceeds the ScalarE cadence; the PSUM accumulator
                # carries across the halves (no other ps_mm user may be
                # emitted between them)
                if half == 0:
                    _khT_pp[(it, nb)] = ps_mm.tile([128, 512], F32,
                                                   tag="mm", name="pp")
                pp = _khT_pp[(it, nb)]
                mts = range(0, 3) if half == 0 else range(3, MT_KV)
                for mt in mts:
                    nc.tensor.matmul(
                        pp[:], wk_b[:, mt, it * 128:(it + 1) * 128],
                        kvT[mt][:, nb * 512:(nb + 1) * 512],
                        start=(mt == 0), stop=(mt == MT_KV - 1))
                if half == 1:
                    nc.vector.tensor_copy(
                        khT[it][:, nb * 512:(nb + 1) * 512], pp[:])

            def emit_khT(it, nb):
                pp = ps_mm.tile([128, 512], F32, tag="mm", name="pp")
                for mt in range(MT_KV):
                    nc.tensor.matmul(
                        pp[:], wk_b[:, mt, it * 128:(it + 1) * 128],
                        kvT[mt][:, nb * 512:(nb + 1) * 512],
                        start=(mt == 0), stop=(mt == MT_KV - 1))
                nc.vector.tensor_copy(khT[it][:, nb * 512:(nb + 1) * 512], pp[:])

            _qhT_pp = {}

            def emit_qhT_half(it, nb, half):
                if half == 0:
                    _qhT_pp[(it, nb)] = ps_mm.tile([128, 512], F32,
                                                   tag="mm", name="pp")
                pp = _qhT_pp[(it, nb)]
                mts = range(0, 2) if half == 0 else range(2, MT_Q)
                for mt in mts:
                    nc.tensor.matmul(
                        pp[:], wq_b[:, mt, it * 128:(it + 1) * 128],
                        qT[mt][:, nb * 512:(nb + 1) * 512],
                        start=(mt == 0), stop=(mt == MT_Q - 1))
                if half == 1:
                    nc.vector.tensor_copy(
                        qhT[it][:, nb * 512:(nb + 1) * 512], pp[:])

            def emit_qhT(it, nb):
                pp = ps_mm.tile([128, 512], F32, tag="mm", name="pp")
                for mt in range(MT_Q):
                    nc.tensor.matmul(
                        pp[:], wq_b[:, mt, it * 128:(it + 1) * 128],
                        qT[mt][:, nb * 512:(nb + 1) * 512],
                        start=(mt == 0), stop=(mt == MT_Q - 1))
                nc.vector.tensor_copy(qhT[it][:, nb * 512:(nb + 1) * 512], pp[:])

            def emit_vh(kt):
                pp = ps_mm.tile([128, INNER_L], F32, tag="mm", name="pp")
                for mt in range(MT_KV):
                    nc.tensor.matmul(
                        pp[:], kvT[mt][:, kt * 128:(kt + 1) * 128],
                        wv_b[:, mt, :],
                        start=(mt == 0), stop=(mt == MT_KV - 1))
                nc.vector.tensor_copy(
                    vh[kt][:, :, 0:DH],
                    pp[:].rearrange("p (h d) -> p h d", h=4))
                nc.vector.tensor_copy(vh[kt][:, :, DH:DA], ones4[:])

            def emit_opart2(nt):
                # first two out-projection partials (own pair 0 + peer pair
                # 0) PSUM-accumulated, bias folded into the single DVE add
                ns = slice(nt * 128, (nt + 1) * 128)
                po = ps_mm.tile([128, 512], F32, tag="mm", name="po")
                nc.tensor.matmul(po[:], attnT[0][:, ns], wo_b[:, 0, :],
                                 start=True, stop=False)
                nc.tensor.matmul(po[:], attnP[0][:, ns], wo_b[:, 2, :],
                                 start=False, stop=True)
                nc.vector.tensor_add(oacc[nt][:], po[:], bo_b[:])

            def emit_final(nt):
                # last two partials (peer pair 1 + own pair 1)
                # PSUM-accumulated + accumulator -> output row tile store
                # (the stride-4 dst AP un-permutes the q-row interleave)
                ns = slice(nt * 128, (nt + 1) * 128)
                po = ps_mm.tile([128, 512], F32, tag="mm", name="po")
                nc.tensor.matmul(po[:], attnP[1][:, ns], wo_b[:, 3, :],
                                 start=True, stop=False)
                nc.tensor.matmul(po[:], attnT[1][:, ns], wo_b[:, 1, :],
                                 start=False, stop=True)
                ot = outs_pool.tile([128, DQ], F32, tag="ot", name="ot")
                nc.vector.tensor_add(ot[:], po[:], oacc[nt][:])
                nc.sync.dma_start(out_d[ns, :], ot[:])

            def emit_export(k, qb):
                # peer-half attnT slice -> DRAM bounce for the AllGather
                nc.sync.dma_start(
                    cc_in[k][qb - 2][:],
                    attnT[k][:, qb * 512:(qb + 1) * 512])

            def emit_cc(k, j):
                nc.gpsimd.collective_compute(
                    "AllGather", mybir.AluOpType.bypass,
                    replica_groups=[[0, 1], [2, 3], [4, 5], [6, 7]],
                    ins=[cc_in[k][j][:].opt()],
                    outs=[cc_out[k][j][:].opt()])

            def emit_import(k, j):
                nc.sync.dma_start(
                    imp[k][:, :, j * 512:(j + 1) * 512],
                    cc_out[k][j][:].rearrange("g p x -> p g x"))

            def emit_blend(k, j):
                # attnP[k] half j = imp[k][member 1-h] via flag = [1-h, h]
                js = slice(j * 512, (j + 1) * 512)
                t1 = blnd_pool.tile([128, 512], BF16, tag="bl", name="t1")
                t2 = blnd_pool.tile([128, 512], BF16, tag="bl", name="t2")
                nc.vector.tensor_scalar_mul(t1[:], imp[k][:, 1, js],
                                            fl_b[:, 0:1])
                nc.vector.tensor_scalar_mul(t2[:], imp[k][:, 0, js],
                                            fl_b[:, 1:2])
                nc.vector.tensor_add(attnP[k][:, js], t1[:], t2[:])

            # pre-attention minimum for block (0, qb=2)
            emit_khT(0, 0)
            emit_qhT(0, 2)
            emit_vh(0)
            emit_vh(1)

            # PE filler queues per block, in block order
            # [(0,2),(0,3),(0,0),(0,1),(1,2),(1,3),(1,0),(1,1)]
            fillers = [
                # (0, qb2): khT(0,*) + all vh + qhT(0,3), ordered so each
                # unit is EMITTED before its first consumer
                ([lambda: emit_khT(0, 1)]
                 + [(lambda kt=kt: emit_vh(kt)) for kt in (2, 3, 4)]
                 + [lambda: emit_khT(0, 2)]
                 + [(lambda kt=kt: emit_vh(kt)) for kt in (5, 6, 7)]
                 + [lambda: emit_khT(0, 3)]
                 + [(lambda kt=kt: emit_vh(kt)) for kt in range(8, KT)]
                 + [lambda: emit_qhT(0, 3)]),
                # (0, qb3)
                ([(lambda b=b: emit_qhT_half(0, 0, b)) for b in (0, 1)]
                 + [(lambda a=a, b=b: emit_khT_half(1, a, b))
                    for a in (0, 1) for b in (0, 1)]),
                # (0, qb0)
                ([(lambda b=b: emit_qhT_half(0, 1, b)) for b in (0, 1)]
                 + [(lambda a=a, b=b: emit_khT_half(1, a, b))
                    for a in (2, 3) for b in (0, 1)]),
                # (0, qb1)
                ([lambda: emit_import(0, 0), lambda: emit_blend(0, 0)]
                 + [(lambda b=b: emit_qhT_half(1, 2, b)) for b in (0, 1)]
                 + [(lambda nt=nt: emit_opart2(nt)) for nt in range(4)]),
                # (1, qb2)
                ([lambda: emit_import(0, 1), lambda: emit_blend(0, 1)]
                 + [(lambda b=b: emit_qhT_half(1, 3, b)) for b in (0, 1)]
                 + [(lambda nt=nt: emit_opart2(nt)) for nt in range(4, 8)]),
                # (1, qb3)
                ([(lambda b=b: emit_qhT_half(1, 0, b)) for b in (0, 1)]),
                # (1, qb0)
                ([lambda: emit_import(1, 0), lambda: emit_blend(1, 0)]
                 + [(lambda b=b: emit_qhT_half(1, 1, b)) for b in (0, 1)]),
                # (1, qb1)
                ([lambda: emit_import(1, 1), lambda: emit_blend(1, 1)]
                 + [(lambda nt=nt: emit_final(nt)) for nt in range(4)]),
            ]

            carry = [None]

            def make_norm(t, qb, pvA, pvB):
                def emit():
                    qs = slice(qb * 512, (qb + 1) * 512)
                    dsb = norm_pool.tile([1, 1024], BF16, tag="nrm", name="dsb")
                    nc.vector.tensor_copy(dsb[0:1, 0:512], pvA[DH:DA, :])
                    nc.vector.tensor_copy(dsb[0:1, 512:1024], pvB[DH:DA, :])
                    dba = ps_mm.tile([64, 512], F32, tag="mm", name="dba")
                    dbb = ps_mm.tile([64, 512], F32, tag="mm", name="dbb")
                    nc.tensor.matmul(dba[:], ones1[:], dsb[0:1, 0:512],
                                     start=True, stop=True)
                    nc.tensor.matmul(dbb[:], ones1[:], dsb[0:1, 512:1024],
                                     start=True, stop=True)
                    rb = norm_pool.tile([64, 1024], F32, tag="nrm", name="rb")
                    nc.vector.reciprocal_approx_fast(rb[:, 0:512], dba[:])
                    nc.vector.reciprocal_approx_fast(rb[:, 512:1024], dbb[:])
                    nc.vector.tensor_mul(attnT[t][0:64, qs],
                                         pvA[0:DH, :], rb[:, 0:512])
                    nc.vector.tensor_mul(attnT[t][64:128, qs],
                                         pvB[0:DH, :], rb[:, 512:1024])
                return emit

            # ---- attention: 2-kt groups, software-pipelined
            # [filler | scores(g) | PV(g-2)]
            blocks = [(t, qb) for t in range(2) for qb in (2, 3, 0, 1)]
            for bi, (t, qb) in enumerate(blocks):
                hA, hB = 2 * t, 2 * t + 1
                qs = slice(qb * 512, (qb + 1) * 512)
                todo = fillers[bi]
                fi = 0

                def emit_scores(kt, t=t, qs=qs):
                    ks = slice(kt * 128, (kt + 1) * 128)
                    sc = ps_sc.tile([128, 1024], F32, tag="sc", name="sc")
                    nc.tensor.matmul(
                        sc[:, 0:512],
                        khT[t][0:64, ks], qhT[t][0:64, qs],
                        start=True, stop=True, tile_position=(0, 0))
                    nc.tensor.matmul(
                        sc[:, 512:1024],
                        khT[t][64:128, ks], qhT[t][64:128, qs],
                        start=True, stop=True, tile_position=(64, 0))
                    ex = exps_pool.tile([128, 1024], BF16, tag="exp",
                                        name="ex")
                    nc.scalar.activation(ex[:], sc[:], EXP,
                                         scale=float(DH) ** -0.5)
                    return ex

                exq = [emit_scores(0), emit_scores(1)]
                if carry[0]:
                    carry[0][0]()      # PV(14) of the previous block
                exq += [emit_scores(2), emit_scores(3)]
                if carry[0]:
                    carry[0][1]()      # PV(15)
                    carry[0][2]()      # normalization (frees old pv tiles)
                    if carry[0][3]:
                        carry[0][3]()  # export + collective (peer-half qbs)
                    carry[0] = None
                pvA = ps_pv.tile([DA, 512], F32, tag="pv", name="pvA")
                pvB = ps_pv.tile([DA, 512], F32, tag="pv", name="pvB")

                def mk_pv(kt, ex, pvA=pvA, pvB=pvB, hA=hA, hB=hB):
                    def emit():
                        nc.tensor.matmul(pvA[:], vh[kt][:, hA, :],
                                         ex[:, 0:512],
                                         start=(kt == 0), stop=(kt == KT - 1))
                        nc.tensor.matmul(pvB[:], vh[kt][:, hB, :],
                                         ex[:, 512:1024],
                                         start=(kt == 0), stop=(kt == KT - 1))
                    return emit

                per_iter = (len(todo) + 5) // 6
                for g in range(2, G):
                    for _ in range(per_iter):
                        if fi < len(todo):
                            todo[fi]()
                            fi += 1
                    exq.append(emit_scores(2 * g))
                    exq.append(emit_scores(2 * g + 1))
                    mk_pv(2 * (g - 2), exq[2 * (g - 2)])()
                    mk_pv(2 * (g - 2) + 1, exq[2 * (g - 2) + 1])()
                mk_pv(KT - 4, exq[KT - 4])()
                mk_pv(KT - 3, exq[KT - 3])()
                while fi < len(todo):
                    todo[fi]()
                    fi += 1
                hook = None
                if qb in (2, 3):
                    def hook(t=t, qb=qb):
                        emit_export(t, qb)
                        emit_cc(t, qb - 2)
                carry[0] = [mk_pv(KT - 2, exq[KT - 2]),
                            mk_pv(KT - 1, exq[KT - 1]),
                            make_norm(t, qb, pvA, pvB), hook]
            carry[0][0]()
            carry[0][1]()
            carry[0][2]()

            # ---- remaining final out-projection rows ----
            for nt in range(4, NT):
                emit_final(nt)

    nc.compile()
    return nc


def _to_bf16(a):
    import ml_dtypes
    return np.ascontiguousarray(np.asarray(a, dtype=np.float32)).astype(
        ml_dtypes.bfloat16)


def make_in_maps(q, kv, Wq, Wk, Wv, Wo, bo):
    q = _to_bf16(q)
    kv = _to_bf16(kv)
    Wq, Wk, Wv, Wo = map(_to_bf16, (Wq, Wk, Wv, Wo))
    bo = np.ascontiguousarray(np.asarray(bo, dtype=np.float32))
    in_maps = []
    for c in range(N_CORES):
        b, h = c // 2, c % 2
        own = slice(h * NQ, (h + 1) * NQ)
        peer = slice((1 - h) * NQ, (2 - h) * NQ)
        cols = slice(h * INNER_L, (h + 1) * INNER_L)
        perm = lambda w, t: np.ascontiguousarray(
            w.reshape(t, 128, w.shape[1]).transpose(1, 0, 2))
        wo_l = np.concatenate([Wo[cols],
                               Wo[slice((1 - h) * INNER_L, (2 - h) * INNER_L)]])
        in_maps.append({
            "q": np.ascontiguousarray(
                np.concatenate([q[b][own], q[b][peer]]).T),
            "kv": np.ascontiguousarray(kv[b].T),
            "Wq": perm(np.ascontiguousarray(Wq[:, cols]), 4),
            "Wk": perm(np.ascontiguousarray(Wk[:, cols]), 6),
            "Wv": perm(np.ascontiguousarray(Wv[:, cols]), 6),
            "Wo": perm(wo_l, 4),
            "bo": bo,
            "flag": np.asarray([1.0 - h, float(h)], dtype=np.float32),
        })
    return in_maps


def kernel(q, kv, Wq, Wk, Wv, Wo, bo):
    from concourse.bass_utils import run_bass_kernel_spmd

    if "nc" not in _cache:
        _cache["nc"] = _build()
    nc = _cache["nc"]

    in_maps = make_in_maps(q, kv, Wq, Wk, Wv, Wo, bo)
    res = run_bass_kernel_spmd(nc, in_maps, core_ids=list(range(N_CORES)))
    out = np.empty((B, NQ_FULL, DQ), dtype=np.float32)
    for c in range(N_CORES):
        b, h = c // 2, c % 2
        out[b, h * NQ:(h + 1) * NQ] = res.results[c]["out"]
    return out
